# revision 2
# baseline (speedup 1.0000x reference)
"""CPSF codebook fused kernel for 8 Trainium2 NeuronCores.

Math (see reference): for each batch row b and codebook entry m,
  q[b,m] = par_sq/s_par + (max(tot_sq-par_sq,0) + max(dd_sq,0))/s_perp
  w[b,m] = alpha[m] * exp(-pi*q)
  out    = Re((w @ (T_hat_re + i*T_hat_im)) @ A.T),  A = exp(i*2pi/S * k*s)

Device strategy (pure batch-parallel, no collectives):
  - The final DFT is folded into the codebook on the host:
      out = w @ TA,   TA = T_hat_re @ cos(ang) - T_hat_im @ sin(ang)
  - Each core handles B/8 = 512 batch rows against the full codebook.
  - The three (b,m) bilinear forms (c_par re/im projections G1/G2 and the
    cross-term sum S) run as fp8e4 DoubleRow matmuls at 0.5 cycles/row,
    2x the fp32r rate. fp8's ~3% quantization error is cancelled to second
    order using the DoubleRow pair slots as error-correction carriers:
      * rhs operands are split hi/lo: pairing weights [W | W/LAM] with
        rhs [r_hi | r_lo] computes W^T(r_hi + r_lo/LAM) = W^T r exactly.
      * the dominant weight-side residual (z_j in L3) rides in the spare
        block of a 4th DoubleRow matmul: [F3/LAM | W4] x [r1_hi | r2_hi].
    Measured end-to-end rel err ~2e-3 (vs 2e-4 full-fp32r, gate 2e-2).
  - w @ TA stays fp32r (fp8 there would cost ~3% output error).
  - Per-core tensor work: 64 tiles x (4 DR matmuls @256cyc + 2 fp32r
    @512cyc) = 128k cycles, vs 197k for the all-fp32r baseline.
"""

import os
import sys

for _p in ("/opt/trn_rl_repo", os.path.expanduser("~/.axon_site/_ro/trn_rl_repo")):
    if os.path.isdir(_p) and _p not in sys.path:
        sys.path.insert(0, _p)

import ml_dtypes
import numpy as np

B, N, M, S = 4096, 64, 8192, 256
NCORES = 8
BLOC = B // NCORES          # 512 batch rows per core
NT = M // 128               # 64 codebook tiles
PI = float(np.pi)
LAM = 16.0                  # hi/lo residual scale (keeps residuals < e4m3 max 240)
E4 = ml_dtypes.float8_e4m3  # TRN fp8e4: max normal 240


def _q8(x):
    return np.asarray(x, E4)


def _deq(x):
    return x.astype(np.float64)


def _pow2scale(x, target=224.0):
    am = float(np.abs(x).max())
    if am == 0.0 or not np.isfinite(am):
        return 1.0
    return float(2.0 ** np.floor(np.log2(target / am)))


def _prep(x_re, x_im, z_j_re, z_j_im, vec_d_j_re, vec_d_j_im,
          T_hat_re, T_hat_im, alpha_j, sigma_par, sigma_perp):
    """Host-side packing/quantization (all O(B*N + M*N + M*S^2) — tiny vs device)."""
    f32 = np.float32
    f64 = np.float64
    tiny = float(np.finfo(f32).tiny)

    # ---- batch side ----
    zr = x_re[:, :N].astype(f64)
    zi = x_im[:, :N].astype(f64)
    vdr = x_re[:, N:].astype(f64)
    vdi = x_im[:, N:].astype(f64)
    nrm = np.sqrt((vdr * vdr + vdi * vdi).sum(-1))
    nrm[nrm == 0] = 1.0
    vdr /= nrm[:, None]
    vdi /= nrm[:, None]
    zsq_b = (zr * zr + zi * zi).sum(-1)
    vdsq_b = (vdr * vdr + vdi * vdi).sum(-1)

    r1 = np.concatenate([zr.T, zi.T], 0)        # [128, B]
    r2 = np.concatenate([vdr.T, vdi.T], 0)      # [128, B]
    sr1 = _pow2scale(r1)
    sr2 = _pow2scale(r2)
    r1h = _q8(r1 * sr1)
    r1l = _q8((r1 * sr1 - _deq(r1h)) * LAM)
    r2h = _q8(r2 * sr2)

    # ---- codebook side ----
    zjr = z_j_re.astype(f64)
    zji = z_j_im.astype(f64)
    vjr = vec_d_j_re.astype(f64)
    vji = vec_d_j_im.astype(f64)
    nj = np.sqrt((vjr * vjr + vji * vji).sum(-1))
    nj[nj == 0] = 1.0
    vjr /= nj[:, None]
    vji /= nj[:, None]

    alpha = np.maximum(alpha_j.astype(f64), tiny)
    s_par = np.maximum(sigma_par.astype(f64), tiny)
    s_perp = np.maximum(sigma_perp.astype(f64), tiny)
    isp = 1.0 / s_perp                           # per-m
    Rm = 1.0 / s_par - isp                       # per-m (negative)

    c0_re = (vjr * zjr + vji * zji).sum(-1)
    c0_im = (vjr * zji - vji * zjr).sum(-1)
    zjsq = (zjr * zjr + zji * zji).sum(-1)
    vjsq = (vjr * vjr + vji * vji).sum(-1)

    # lhsT stacks [K=128, M]
    L1 = np.concatenate([vjr.T, vji.T], 0)
    L2 = np.concatenate([-vji.T, vjr.T], 0)
    L3 = np.concatenate([zjr.T, zji.T], 0) * (-2.0 * isp)[None, :]
    L4 = np.concatenate([vjr.T, vji.T], 0) * (-2.0 * isp)[None, :]

    s1 = _pow2scale(np.concatenate([L1, L2], 0))
    s3 = _pow2scale(L3)
    sigma_s = s3 * sr1                 # PSUM scale of the S accumulator
    s4 = sigma_s / sr2
    if np.abs(L4).max() * s4 > 224.0:  # keep W4 in range for adversarial inputs
        s4 = _pow2scale(L4)
        sigma_s = s4 * sr2
        s3 = sigma_s / sr1
    W1 = _q8(L1 * s1)
    W1b = _q8(_deq(W1) / LAM)
    W2 = _q8(L2 * s1)
    W2b = _q8(_deq(W2) / LAM)
    W3 = _q8(L3 * s3)
    W3b = _q8(_deq(W3) / LAM)
    F3 = _q8((L3 * s3 - _deq(W3)) * LAM)
    F3b = _q8(_deq(F3) / LAM)
    W4 = _q8(L4 * s4)

    # weight pack per m-tile: wq[t, k, slot, m] slots = W1,W1b,W2,W2b,W3,W3b,F3b,W4
    wq = np.empty((NT, 128, 8, 128), E4)
    for t in range(NT):
        sl = slice(t * 128, (t + 1) * 128)
        for j, Wm in enumerate((W1, W1b, W2, W2b, W3, W3b, F3b, W4)):
            wq[t, :, j, :] = Wm[:, sl]

    # per-m epilogue scalars, tile-packed: mv[p, t*4+j]
    b1 = (-c0_re).astype(f32)                    # ACT square bias for G1
    b2 = (-c0_im).astype(f32)
    be = (np.log(alpha) - PI * (zjsq + vjsq) * isp).astype(f32)
    mv3 = (Rm * sigma_s).astype(f32)             # DVE stt scalar
    mv = np.empty((128, NT * 4), f32)
    for t in range(NT):
        sl = slice(t * 128, (t + 1) * 128)
        mv[:, t * 4 + 0] = b1[sl]
        mv[:, t * 4 + 1] = b2[sl]
        mv[:, t * 4 + 2] = be[sl]
        mv[:, t * 4 + 3] = mv3[sl]

    # DFT folded into the codebook: TA = T_hat_re @ cos - T_hat_im @ sin.
    nn = np.arange(S, dtype=f32)
    ang = f32(2.0 * np.pi / S) * (nn[:, None] * nn[None, :])
    cosA = np.cos(ang).astype(f32)
    sinA = np.sin(ang).astype(f32)
    TA = (T_hat_re.astype(f64) @ cosA.astype(f64)
          - T_hat_im.astype(f64) @ sinA.astype(f64)).astype(f32)
    TA = np.ascontiguousarray(TA.reshape(NT, 128, S))

    # the (zsq_b + vdsq_b)*isp term: split isp = c0v + delta; the c0v part
    # factors out of the exp as an exact per-b output row scale; a nonzero
    # delta needs the rank-1 matmul on device (never for these inputs).
    c0v = float(isp.mean())
    delta = (isp - c0v).astype(f32)
    uniform = bool(np.all(delta == 0))
    vraw = zsq_b + vdsq_b
    erow = np.exp(-PI * c0v * vraw).astype(f32)          # [B] output row scale
    osc = np.ascontiguousarray(erow.reshape(NCORES, 4, 128).transpose(0, 2, 1))
    vrow = vraw.astype(f32)[None, :]                     # [1, B]

    scal = (float(1.0 / (s1 * sr1)), float(sigma_s))     # (inv1, sigma_s)
    return dict(r1h=r1h, r1l=r1l, r2h=r2h, wq=wq, ta=TA, mv=mv,
                ispd=np.ascontiguousarray((delta * f32(sigma_s))[None, :]),
                vrow=vrow, osc=osc, uniform=uniform, scal=scal)


_CACHED = {}


def _build_nc(uniform, inv1, sigma_s):
    key = ("nc", uniform, inv1, sigma_s)
    if key in _CACHED:
        return _CACHED[key]
    import concourse.bacc as bacc
    import concourse.masks as masks
    import concourse.mybir as mybir
    import concourse.tile as tile

    F32 = mybir.dt.float32
    F32R = mybir.dt.float32r
    F8 = mybir.dt.float8e4
    AF = mybir.ActivationFunctionType
    OP = mybir.AluOpType
    DR = mybir.MatmulPerfMode.DoubleRow

    nc = bacc.Bacc("TRN2", target_bir_lowering=False, debug=False,
                   num_devices=NCORES)
    d_rA = nc.dram_tensor("rhsA", [128, 2, BLOC], F8, kind="ExternalInput").ap()
    d_rC = nc.dram_tensor("rhsC", [128, 2, BLOC], F8, kind="ExternalInput").ap()
    d_wq = nc.dram_tensor("wq", [NT, 128, 8, 128], F8, kind="ExternalInput").ap()
    d_ta = nc.dram_tensor("ta", [NT, 128, S], F32R, kind="ExternalInput").ap()
    d_mv = nc.dram_tensor("mv", [128, NT * 4], F32, kind="ExternalInput").ap()
    d_isp = nc.dram_tensor("ispd", [1, M], F32R, kind="ExternalInput").ap()
    d_v = nc.dram_tensor("vrow", [1, BLOC], F32R, kind="ExternalInput").ap()
    d_osc = nc.dram_tensor("osc", [128, 4], F32, kind="ExternalInput").ap()
    d_out = nc.dram_tensor("out", [BLOC, S], F32, kind="ExternalOutput").ap()

    with tile.TileContext(nc) as tc:
        with tc.tile_pool(name="const", bufs=1) as cp, \
             tc.tile_pool(name="lp", bufs=10) as lpool, \
             tc.tile_pool(name="g", bufs=2, space="PSUM") as gpool, \
             tc.tile_pool(name="tacc", bufs=1, space="PSUM") as taccp, \
             tc.tile_pool(name="u", bufs=6) as upool, \
             tc.tile_pool(name="w", bufs=6) as wpool:
            rA = cp.tile([128, 2, BLOC], F8)
            rC = cp.tile([128, 2, BLOC], F8)
            mv = cp.tile([128, NT * 4], F32)
            osc = cp.tile([128, 4], F32)
            ident = cp.tile([128, 128], F32)
            nc.sync.dma_start(rA[:], d_rA)
            nc.sync.dma_start(rC[:], d_rC)
            nc.sync.dma_start(mv[:], d_mv)
            nc.sync.dma_start(osc[:], d_osc)
            masks.make_identity(nc, ident[:])
            if not uniform:
                isp = cp.tile([1, M], F32R)
                vrow = cp.tile([1, BLOC], F32R)
                nc.sync.dma_start(isp[:], d_isp)
                nc.sync.dma_start(vrow[:], d_v)

            # out.T accumulators: [128 k-half, 512 b]
            ot0 = taccp.tile([128, BLOC], F32)
            ot1 = taccp.tile([128, BLOC], F32)

            for t in range(NT):
                wqt = lpool.tile([128, 8, 128], F8, tag="wq")
                ta = lpool.tile([128, S], F32R, tag="ta")
                nc.sync.dma_start(wqt[:], d_wq[t])
                nc.sync.dma_start(ta[:], d_ta[t])

                g1 = gpool.tile([128, BLOC], F32, tag="g1")
                g2 = gpool.tile([128, BLOC], F32, tag="g2")
                s = gpool.tile([128, BLOC], F32, tag="s")
                nc.tensor.matmul(g1[:], wqt[:, 0:2, :], rA[:],
                                 start=True, stop=True, perf_mode=DR)
                nc.tensor.matmul(g2[:], wqt[:, 2:4, :], rA[:],
                                 start=True, stop=True, perf_mode=DR)
                nc.tensor.matmul(s[:], wqt[:, 4:6, :], rA[:],
                                 start=True, stop=False, perf_mode=DR)
                nc.tensor.matmul(s[:], wqt[:, 6:8, :], rC[:],
                                 start=False, stop=uniform, perf_mode=DR)
                if not uniform:
                    nc.tensor.matmul(s[:], isp[:, t * 128:(t + 1) * 128], vrow[:],
                                     start=False, stop=True)

                u1 = upool.tile([128, BLOC], F32, tag="u1")
                u2 = upool.tile([128, BLOC], F32, tag="u2")
                nc.scalar.activation(u1[:], g1[:], AF.Square,
                                     bias=mv[:, t * 4:t * 4 + 1], scale=inv1)
                nc.scalar.activation(u2[:], g2[:], AF.Square,
                                     bias=mv[:, t * 4 + 1:t * 4 + 2], scale=inv1)
                u = upool.tile([128, BLOC], F32, tag="u")
                nc.vector.tensor_add(u[:], u1[:], u2[:])
                # q (sigma_s-scaled) written in-place into the S PSUM bank
                nc.vector.scalar_tensor_tensor(
                    s[:], u[:], mv[:, t * 4 + 3:t * 4 + 4], s[:],
                    op0=OP.mult, op1=OP.add)
                w = wpool.tile([128, BLOC], F32R, tag="w")
                nc.scalar.activation(w[:], s[:], AF.Exp,
                                     bias=mv[:, t * 4 + 2:t * 4 + 3],
                                     scale=-PI / sigma_s)

                # out.T[k, b] += ta[m, k].T @ w[m, b]
                for h, oth in ((0, ot0), (1, ot1)):
                    nc.tensor.matmul(oth[:], ta[:, h * 128:(h + 1) * 128],
                                     w[:], start=(t == 0), stop=(t == NT - 1),
                                     skip_group_check=True)

            # transpose out.T -> out, scaling rows by osc, then DMA out
            obs = [wpool.tile([128, S], F32, tag=f"ob{j}", bufs=1,
                              name=f"ob{j}") for j in range(4)]
            for h, oth in ((0, ot0), (1, ot1)):
                tsb = wpool.tile([128, BLOC], F32, tag="tsb")
                nc.scalar.copy(tsb[:], oth[:])
                for j in range(4):
                    pt = gpool.tile([128, 128], F32, tag="g1", name=f"pt{h}{j}")
                    nc.tensor.transpose(pt[:], tsb[:, j * 128:(j + 1) * 128],
                                        ident[:])
                    nc.vector.tensor_scalar_mul(
                        obs[j][:, h * 128:(h + 1) * 128], pt[:], osc[:, j:j + 1])
            for j in range(4):
                nc.sync.dma_start(d_out[j * 128:(j + 1) * 128, :], obs[j][:])
    nc.compile()
    _CACHED[key] = nc
    return nc


def _run(inputs, trace=False):
    from concourse.bass_utils import run_bass_kernel_spmd

    prep = _prep(**inputs)
    inv1, sigma_s = prep["scal"]
    nc = _build_nc(prep["uniform"], inv1, sigma_s)
    shared = {k: prep[k] for k in ("wq", "ta", "mv", "ispd")}
    in_maps = []
    for c in range(NCORES):
        sl = slice(c * BLOC, (c + 1) * BLOC)
        rhsA = np.ascontiguousarray(
            np.stack([prep["r1h"][:, sl], prep["r1l"][:, sl]], axis=1))
        rhsC = np.ascontiguousarray(
            np.stack([prep["r1h"][:, sl], prep["r2h"][:, sl]], axis=1))
        in_maps.append(dict(rhsA=rhsA, rhsC=rhsC,
                            vrow=np.ascontiguousarray(prep["vrow"][:, sl]),
                            osc=np.ascontiguousarray(prep["osc"][c]),
                            **shared))
    res = run_bass_kernel_spmd(nc, in_maps, list(range(NCORES)), trace=trace)
    out = np.concatenate([res.results[c]["out"] for c in range(NCORES)], 0)
    return out.astype(np.float32), res


def kernel(**inputs):
    out, _ = _run(inputs, trace=False)
    return out


def _install_ntff_hook():
    """The agent image's antenv lacks axon_hooks; recreate it so trace=True
    can capture NTFF profiles via libaxon_pjrt.so (same mechanism as
    trn_agent_boot.trn_boot)."""
    import types

    try:
        from antenv.axon_hooks import get_axon_ntff_profile_hook  # noqa: F401
        return
    except ImportError:
        pass
    import contextlib
    import ctypes

    so_path = "/opt/axon/libaxon_pjrt.so"
    lib = ctypes.CDLL(so_path)
    lib.axon_start_nrt_profile.argtypes = [ctypes.POINTER(ctypes.c_int64),
                                           ctypes.c_size_t]
    lib.axon_start_nrt_profile.restype = ctypes.c_int64
    lib.axon_stop_nrt_profile.argtypes = [ctypes.c_char_p]
    lib.axon_stop_nrt_profile.restype = ctypes.c_int64

    @contextlib.contextmanager
    def _hook(output_dir, device_ids):
        import jax

        jax.devices()
        if device_ids:
            ids = (ctypes.c_int64 * len(device_ids))(*device_ids)
            rc = lib.axon_start_nrt_profile(ids, len(device_ids))
        else:
            rc = lib.axon_start_nrt_profile(None, 0)
        if rc != 0:
            raise RuntimeError(f"axon_start_nrt_profile rc={rc}")
        try:
            yield
        finally:
            n = lib.axon_stop_nrt_profile(str(output_dir).encode())
            if n < 0:
                raise RuntimeError(f"axon_stop_nrt_profile rc={n}")
            if n == 0:
                print("WARNING: NTFF capture wrote nothing (raced the execute)")

    mod = types.ModuleType("antenv.axon_hooks")
    mod.get_axon_ntff_profile_hook = lambda: _hook
    mod.set_axon_ntff_profile_hook = lambda h: None
    sys.modules["antenv.axon_hooks"] = mod
    import antenv

    antenv.axon_hooks = mod


def run_traced(inputs):
    _install_ntff_hook()
    return _run(inputs, trace=True)


# revision 9
# speedup vs baseline: 1.1766x; 1.1766x over previous
"""CPSF codebook fused kernel for 8 Trainium2 NeuronCores.

Math (see reference): for each batch row b and codebook entry m,
  q[b,m] = par_sq/s_par + (max(tot_sq-par_sq,0) + max(dd_sq,0))/s_perp
  w[b,m] = alpha[m] * exp(-pi*q)
  out    = Re((w @ (T_hat_re + i*T_hat_im)) @ A.T),  A = exp(i*2pi/S * k*s)

Device strategy (pure batch-parallel, no collectives), per core B/8 = 512
batch rows against the full codebook, 64 m-tiles of 128:
  - The final DFT, the per-m constants (ln alpha - pi*K_m), and the per-b
    constants (row scale osc) are folded on the host; the exp()'s bias and
    the squares' biases are eliminated by expanding |c_par - c0|^2 and
    folding the linear c0-term into the S-matmul weights. This leaves a
    5-op elementwise epilogue per tile with no per-m operands:
      u1 = Square(sq_scale*g1)      [ACT]
      u2 = g2*g2                    [DVE]
      v  = u2*kappa + u1            [GpSimd stt]
      x  = v + s                    [DVE, x->SBUF so s frees early]
      w~ = Exp(exp_scale*x)         [ACT]
    software-pipelined (x/exp skewed one tile, out-matmuls two tiles).
  - q-side bilinear forms run as fp8e4 DoubleRow matmuls (2x MACs/instr:
    each instruction sums two K=128 products). fp8 quantization error is
    cancelled to second order using the pair slots as correction carriers:
    weights [W | W/LAM] against rhs [r_hi | r_lo] give W^T r exactly in
    the rhs; the dominant weight-side residual (F3/LAM) rides in the
    spare slot of the 4th instruction. End-to-end rel err ~2e-3 (gate 2e-2).
  - out[b,s] is accumulated DIRECTLY (4 fp32r quarter-matmuls per tile,
    lhsT = w-slice, rhs = TA'), so no transpose epilogue is needed.
"""

import os
import sys

for _p in ("/opt/trn_rl_repo", os.path.expanduser("~/.axon_site/_ro/trn_rl_repo")):
    if os.path.isdir(_p) and _p not in sys.path:
        sys.path.insert(0, _p)

import ml_dtypes
import numpy as np

B, N, M, S = 4096, 64, 8192, 256
NCORES = 8
BLOC = B // NCORES          # 512 batch rows per core
NT = M // 128               # 64 codebook tiles
PI = float(np.pi)
LAM = 16.0                  # hi/lo residual scale (keeps residuals < e4m3 max 240)
E4 = ml_dtypes.float8_e4m3  # TRN fp8e4: max normal 240


def _q8(x):
    return np.asarray(x, E4)


def _deq(x):
    return x.astype(np.float64)


def _pow2scale(x, target=224.0):
    am = float(np.abs(x).max())
    if am == 0.0 or not np.isfinite(am):
        return 1.0
    return float(2.0 ** np.floor(np.log2(target / am)))


def _prep(x_re, x_im, z_j_re, z_j_im, vec_d_j_re, vec_d_j_im,
          T_hat_re, T_hat_im, alpha_j, sigma_par, sigma_perp):
    """Host-side packing/quantization (all O(B*N + M*N + M*S^2) — tiny vs device)."""
    f32 = np.float32
    f64 = np.float64
    tiny = float(np.finfo(f32).tiny)

    # ---- batch side ----
    zr = x_re[:, :N].astype(f64)
    zi = x_im[:, :N].astype(f64)
    vdr = x_re[:, N:].astype(f64)
    vdi = x_im[:, N:].astype(f64)
    nrm = np.sqrt((vdr * vdr + vdi * vdi).sum(-1))
    nrm[nrm == 0] = 1.0
    vdr /= nrm[:, None]
    vdi /= nrm[:, None]
    zsq_b = (zr * zr + zi * zi).sum(-1)
    vdsq_b = (vdr * vdr + vdi * vdi).sum(-1)

    r1 = np.concatenate([zr.T, zi.T], 0)        # [128, B]
    r2 = np.concatenate([vdr.T, vdi.T], 0)      # [128, B]
    sr1 = _pow2scale(r1)
    sr2 = _pow2scale(r2)
    r1h = _q8(r1 * sr1)
    r1l = _q8((r1 * sr1 - _deq(r1h)) * LAM)
    r2h = _q8(r2 * sr2)

    # ---- codebook side ----
    zjr = z_j_re.astype(f64)
    zji = z_j_im.astype(f64)
    vjr = vec_d_j_re.astype(f64)
    vji = vec_d_j_im.astype(f64)
    nj = np.sqrt((vjr * vjr + vji * vji).sum(-1))
    nj[nj == 0] = 1.0
    vjr /= nj[:, None]
    vji /= nj[:, None]

    alpha = np.maximum(alpha_j.astype(f64), tiny)
    s_par = np.maximum(sigma_par.astype(f64), tiny)
    s_perp = np.maximum(sigma_perp.astype(f64), tiny)
    isp = 1.0 / s_perp                           # per-m
    Rm = 1.0 / s_par - isp                       # per-m (negative normally)
    use_mv = bool(np.any(Rm > 0))

    c0_re = (vjr * zjr + vji * zji).sum(-1)
    c0_im = (vjr * zji - vji * zjr).sum(-1)
    zjsq = (zjr * zjr + zji * zji).sum(-1)
    vjsq = (vjr * vjr + vji * vji).sum(-1)

    # projection stacks [K=128, M]
    L1 = np.concatenate([vjr.T, vji.T], 0)
    L2 = np.concatenate([-vji.T, vjr.T], 0)
    # S weights (sign-flipped so x = -sigma_s*Qvar): w3 = 2[R(c0re*L1+c0im*L2)
    # + isp*zj-stack], w4 = 2*isp*vdj-stack; the c0-linear term of the
    # expanded |c_par - c0|^2 is folded in (vanishes when use_mv).
    lin = 0.0 if use_mv else Rm
    w3 = 2.0 * (lin * (c0_re[None, :] * L1 + c0_im[None, :] * L2)
                + isp[None, :] * np.concatenate([zjr.T, zji.T], 0))
    w4 = 2.0 * isp[None, :] * np.concatenate([vjr.T, vji.T], 0)

    # largest pow2 sigma_s keeping both W3 and W4 inside e4m3 range
    sigma_s = min(_pow2scale(w3) * sr1, _pow2scale(w4) * sr2)
    W3 = _q8(w3 * (sigma_s / sr1))
    W3b = _q8(_deq(W3) / LAM)
    F3 = _q8((w3 * (sigma_s / sr1) - _deq(W3)) * LAM)
    F3b = _q8(_deq(F3) / LAM)
    W4 = _q8(w4 * (sigma_s / sr2))

    c1 = _pow2scale(np.concatenate([L1, L2], 0))
    W1 = _q8(L1 * c1)
    W1b = _q8(_deq(W1) / LAM)
    W2 = _q8(L2 * c1)
    W2b = _q8(_deq(W2) / LAM)
    # epilogue scalars: u1 must equal sigma_s*|R|*Pre^2 (uniform R) else raw
    g_scale = c1 * sr1                      # psum scale of G1/G2
    if use_mv:
        sq_scale = 1.0                      # u raw; per-m mv applies -sigma_s*R
        kappa = 1.0
        mv3 = (-Rm * sigma_s / (g_scale * g_scale)).astype(f32)
    else:
        R0 = float(Rm[0]) if np.all(Rm == Rm[0]) else None
        # per-m R<=0 but non-constant still needs mv; detect
        if R0 is None or not np.all(Rm == Rm[0]):
            use_mv = True
            sq_scale = 1.0
            kappa = 1.0
            mv3 = (-Rm * sigma_s / (g_scale * g_scale)).astype(f32)
        else:
            sq_scale = float(np.sqrt(-R0 * sigma_s) / g_scale)
            kappa = float(-R0 * sigma_s / (g_scale * g_scale))
            mv3 = np.zeros(M, f32)
    mv = np.empty((128, NT), f32)
    for t in range(NT):
        mv[:, t] = mv3[t * 128:(t + 1) * 128]

    # weight pack per m-tile: wq[t, k, slot, m]
    wq = np.empty((NT, 128, 8, 128), E4)
    for t in range(NT):
        sl = slice(t * 128, (t + 1) * 128)
        for j, Wm in enumerate((W1, W1b, W2, W2b, W3, W3b, F3b, W4)):
            wq[t, :, j, :] = Wm[:, sl]

    # DFT folded into the codebook, then per-m constants folded into TA:
    # TA'[m,s] = alpha_m * exp(-pi*K_m) * TA[m,s],
    # K_m = R|c0|^2 (if folded) + (zjsq+vjsq)*isp
    nn = np.arange(S, dtype=f32)
    ang = f32(2.0 * np.pi / S) * (nn[:, None] * nn[None, :])
    cosA = np.cos(ang).astype(f32)
    sinA = np.sin(ang).astype(f32)
    TA = (T_hat_re.astype(f64) @ cosA.astype(f64)
          - T_hat_im.astype(f64) @ sinA.astype(f64))
    Km = lin * (c0_re * c0_re + c0_im * c0_im) + (zjsq + vjsq) * isp
    TA *= (alpha * np.exp(-PI * Km))[:, None]
    TA = np.ascontiguousarray(TA.astype(f32).reshape(NT, 128, S))

    # the (zsq_b + vdsq_b)*isp term: isp = c0v + delta; c0v -> exact per-b
    # output row scale; nonzero delta -> rank-1 matmul (never for these inputs)
    if np.all(isp == isp[0]):
        c0v = float(isp[0])
    else:
        c0v = float(isp.mean())
    delta = isp - c0v
    rank1 = bool(np.any(delta != 0))
    vraw = zsq_b + vdsq_b
    erow = np.exp(-PI * c0v * vraw).astype(f32)          # [B] output row scale
    osc = np.ascontiguousarray(erow.reshape(NCORES, 4, 128).transpose(0, 2, 1))
    vrow = vraw.astype(f32)[None, :]                     # [1, B]

    scal = (use_mv, rank1, sq_scale, kappa, float(PI / sigma_s))
    return dict(r1h=r1h, r1l=r1l, r2h=r2h, wq=wq, ta=TA, mv=mv,
                ispd=np.ascontiguousarray((-delta * sigma_s).astype(f32)[None, :]),
                vrow=vrow, osc=osc, scal=scal)


_CACHED = {}


def _build_nc(use_mv, rank1, sq_scale, kappa, exp_scale):
    key = ("nc", use_mv, rank1, sq_scale, kappa, exp_scale)
    if key in _CACHED:
        return _CACHED[key]
    import concourse.bacc as bacc
    import concourse.mybir as mybir
    import concourse.tile as tile

    F32 = mybir.dt.float32
    F32R = mybir.dt.float32r
    F8 = mybir.dt.float8e4
    AF = mybir.ActivationFunctionType
    OP = mybir.AluOpType
    DR = mybir.MatmulPerfMode.DoubleRow

    nc = bacc.Bacc("TRN2", target_bir_lowering=False, debug=False,
                   num_devices=NCORES)
    d_rA = nc.dram_tensor("rhsA", [128, 2, BLOC], F8, kind="ExternalInput").ap()
    d_rC = nc.dram_tensor("rhsC", [128, 2, BLOC], F8, kind="ExternalInput").ap()
    d_wq = nc.dram_tensor("wq", [NT, 128, 8, 128], F8, kind="ExternalInput").ap()
    d_ta = nc.dram_tensor("ta", [NT, 128, S], F32R, kind="ExternalInput").ap()
    d_mv = nc.dram_tensor("mv", [128, NT], F32, kind="ExternalInput").ap()
    d_isp = nc.dram_tensor("ispd", [1, M], F32R, kind="ExternalInput").ap()
    d_v = nc.dram_tensor("vrow", [1, BLOC], F32R, kind="ExternalInput").ap()
    d_osc = nc.dram_tensor("osc", [128, 4], F32, kind="ExternalInput").ap()
    d_out = nc.dram_tensor("out", [BLOC, S], F32, kind="ExternalOutput").ap()

    with tile.TileContext(nc) as tc:
        with tc.tile_pool(name="const", bufs=1) as cp, \
             tc.tile_pool(name="lp", bufs=10) as lpool, \
             tc.tile_pool(name="g", bufs=2, space="PSUM") as gpool, \
             tc.tile_pool(name="oacc", bufs=1, space="PSUM") as opool, \
             tc.tile_pool(name="u", bufs=3) as upool, \
             tc.tile_pool(name="w", bufs=5) as wpool:
            rA = cp.tile([128, 2, BLOC], F8)
            rC = cp.tile([128, 2, BLOC], F8)
            osc = cp.tile([128, 4], F32)
            # chunked loads so the first matmul isn't gated on one DMA queue
            for ch in range(2):
                csl = slice(ch * 256, (ch + 1) * 256)
                nc.sync.dma_start(rA[:, :, csl], d_rA[:, :, csl])
                nc.sync.dma_start(rC[:, :, csl], d_rC[:, :, csl])
            nc.sync.dma_start(osc[:], d_osc)
            if use_mv:
                mv = cp.tile([128, NT], F32)
                nc.sync.dma_start(mv[:], d_mv)
            if rank1:
                isp = cp.tile([1, M], F32R)
                vrow = cp.tile([1, BLOC], F32R)
                nc.sync.dma_start(isp[:], d_isp)
                nc.sync.dma_start(vrow[:], d_v)

            # direct out accumulators: 4 batch-quarters x [128 b, 256 s].
            # Quarters share PSUM banks, so a start=True zero-region would
            # wipe the bank-mate quarter: memset once, accumulate always.
            oq = opool.tile([128, 4, S], F32)
            nc.vector.memset(oq[:], 0.0)

            tiles = {}
            def issue_out(tj):
                wj, half, taj = tiles.pop(tj)
                for qd in range(4):
                    nc.tensor.matmul(
                        oq[:, qd, :],
                        wj[:, half, qd * 128:(qd + 1) * 128],
                        taj[:], start=False, stop=(tj == NT - 1),
                        skip_group_check=True)

            xp = wp = None
            for t in range(NT):
                wqt = lpool.tile([128, 8, 128], F8, tag="wq")
                ta = lpool.tile([128, S], F32R, tag="ta")
                if t < 2:
                    for ch in range(2):
                        nc.sync.dma_start(wqt[:, ch * 4:(ch + 1) * 4, :],
                                          d_wq[t][:, ch * 4:(ch + 1) * 4, :])
                        nc.sync.dma_start(ta[:, ch * 128:(ch + 1) * 128],
                                          d_ta[t][:, ch * 128:(ch + 1) * 128])
                else:
                    nc.sync.dma_start(wqt[:], d_wq[t])
                    nc.sync.dma_start(ta[:], d_ta[t])

                # G1/G2 into halves of one 2-bank PSUM tile -> one ACT square
                g12 = gpool.tile([128, 2, BLOC], F32, tag="g12")
                s = gpool.tile([128, BLOC], F32, tag="s")
                nc.tensor.matmul(g12[:, 0, :], wqt[:, 0:2, :], rA[:],
                                 start=True, stop=True, perf_mode=DR)
                nc.tensor.matmul(g12[:, 1, :], wqt[:, 2:4, :], rA[:],
                                 start=True, stop=True, perf_mode=DR)
                if t >= 2:
                    issue_out(t - 2)
                nc.tensor.matmul(s[:], wqt[:, 4:6, :], rA[:],
                                 start=True, stop=False, perf_mode=DR)
                nc.tensor.matmul(s[:], wqt[:, 6:8, :], rC[:],
                                 start=False, stop=not rank1, perf_mode=DR)
                if rank1:
                    nc.tensor.matmul(s[:], isp[:, t * 128:(t + 1) * 128], vrow[:],
                                     start=False, stop=True)

                u12 = upool.tile([128, 2, BLOC], F32, tag="u12")
                nc.scalar.activation(u12[:], g12[:], AF.Square, scale=sq_scale)
                v = upool.tile([128, BLOC], F32, tag="v")
                nc.vector.tensor_add(v[:], u12[:, 0, :], u12[:, 1, :])
                if t % 2 == 0:
                    xp = upool.tile([128, 2, BLOC], F32, tag="xp",
                                    name=f"xp{t}")
                if use_mv:
                    nc.vector.scalar_tensor_tensor(
                        xp[:, t % 2, :], v[:], mv[:, t:t + 1], s[:],
                        op0=OP.mult, op1=OP.add)
                else:
                    nc.vector.tensor_add(xp[:, t % 2, :], v[:], s[:])
                tiles[t] = ta
                if t % 2 == 1:
                    wp = wpool.tile([128, 2, BLOC], F32R, tag="wp",
                                    name=f"wp{t}")
                    nc.scalar.activation(wp[:], xp[:], AF.Exp, scale=exp_scale)
                    tiles[t - 1] = (wp, 0, tiles[t - 1])
                    tiles[t] = (wp, 1, tiles[t])
            issue_out(NT - 2)
            issue_out(NT - 1)

            # scale rows by osc and store
            for qd in range(4):
                ob = wpool.tile([128, S], F32, tag=f"ob{qd}", bufs=1,
                                name=f"ob{qd}")
                nc.vector.tensor_scalar_mul(ob[:], oq[:, qd, :],
                                            osc[:, qd:qd + 1])
                nc.sync.dma_start(d_out[qd * 128:(qd + 1) * 128, :], ob[:])
    nc.compile()
    _CACHED[key] = nc
    return nc


def _run(inputs, trace=False):
    from concourse.bass_utils import run_bass_kernel_spmd

    prep = _prep(**inputs)
    nc = _build_nc(*prep["scal"])
    shared = {k: prep[k] for k in ("wq", "ta", "mv", "ispd")}
    in_maps = []
    for c in range(NCORES):
        sl = slice(c * BLOC, (c + 1) * BLOC)
        rhsA = np.ascontiguousarray(
            np.stack([prep["r1h"][:, sl], prep["r1l"][:, sl]], axis=1))
        rhsC = np.ascontiguousarray(
            np.stack([prep["r1h"][:, sl], prep["r2h"][:, sl]], axis=1))
        in_maps.append(dict(rhsA=rhsA, rhsC=rhsC,
                            vrow=np.ascontiguousarray(prep["vrow"][:, sl]),
                            osc=np.ascontiguousarray(prep["osc"][c]),
                            **shared))
    res = run_bass_kernel_spmd(nc, in_maps, list(range(NCORES)), trace=trace)
    out = np.concatenate([res.results[c]["out"] for c in range(NCORES)], 0)
    return out.astype(np.float32), res


def kernel(**inputs):
    out, _ = _run(inputs, trace=False)
    return out


def _install_ntff_hook():
    """The agent image's antenv lacks axon_hooks; recreate it so trace=True
    can capture NTFF profiles via libaxon_pjrt.so (same mechanism as
    trn_agent_boot.trn_boot)."""
    import types

    try:
        from antenv.axon_hooks import get_axon_ntff_profile_hook  # noqa: F401
        return
    except ImportError:
        pass
    import contextlib
    import ctypes

    so_path = "/opt/axon/libaxon_pjrt.so"
    lib = ctypes.CDLL(so_path)
    lib.axon_start_nrt_profile.argtypes = [ctypes.POINTER(ctypes.c_int64),
                                           ctypes.c_size_t]
    lib.axon_start_nrt_profile.restype = ctypes.c_int64
    lib.axon_stop_nrt_profile.argtypes = [ctypes.c_char_p]
    lib.axon_stop_nrt_profile.restype = ctypes.c_int64

    @contextlib.contextmanager
    def _hook(output_dir, device_ids):
        import jax

        jax.devices()
        if device_ids:
            ids = (ctypes.c_int64 * len(device_ids))(*device_ids)
            rc = lib.axon_start_nrt_profile(ids, len(device_ids))
        else:
            rc = lib.axon_start_nrt_profile(None, 0)
        if rc != 0:
            raise RuntimeError(f"axon_start_nrt_profile rc={rc}")
        try:
            yield
        finally:
            n = lib.axon_stop_nrt_profile(str(output_dir).encode())
            if n < 0:
                raise RuntimeError(f"axon_stop_nrt_profile rc={n}")
            if n == 0:
                print("WARNING: NTFF capture wrote nothing (raced the execute)")

    mod = types.ModuleType("antenv.axon_hooks")
    mod.get_axon_ntff_profile_hook = lambda: _hook
    mod.set_axon_ntff_profile_hook = lambda h: None
    sys.modules["antenv.axon_hooks"] = mod
    import antenv

    antenv.axon_hooks = mod


def run_traced(inputs):
    _install_ntff_hook()
    return _run(inputs, trace=True)
